# revision 2
# baseline (speedup 1.0000x reference)
"""v4 Trainium2 Bass kernel for nn_CSM_62216896250023 (dense_cnn).

One merged sigmoid ACT instruction per layer per tile (7-8 ACT/tile).
Single SWDGE gather per tile from a host-prescaled table (emb * pivot
tap of conv1). Conv taps applied with pair-broadcast tensor_tensor
(2x DVE fast mode, per-channel constants as packed [.,2] pairs):
  L1: a = x'[mult]*S1 (tt), Z1 = a += x'[add] (tt, in-place)
  L2: m = Y1[mult]*S2, m += Y1[add], Z2 = m *= Kpivot (DVE j0-3,
      Pool j4-7)
  L3: PE true-diag 3-tap chains, psum groups {2,2,2,2}
  L4: PE true-diag chains -> one psum group
All fp16; output fp16 -> host fp32.
"""
import numpy as np

import concourse.bacc as bacc
import concourse.tile as tile
import concourse.bass_utils as bass_utils
from concourse import mybir

VOCAB, EMBED, BATCH, SEQ = 32000, 1024, 16384, 7
NCORES = 8
BCORE = BATCH // NCORES          # 2048
BT = 256
NT = BCORE // BT                 # 8 tiles
JB = EMBED // 128                # 8
NIDX = SEQ * BT                  # 1792
SCOLS = NIDX // 16               # 112
NDIAG = JB * 3 * 2               # 48 (L3 + L4)
POOL_KLAST_J0 = 4                # j4-7 L2 k-pivot mult on Pool

_prog_cache = {}


def _build_program():
    if "nc" in _prog_cache:
        return _prog_cache["nc"]
    f32, f16, i16 = mybir.dt.float32, mybir.dt.float16, mybir.dt.int16
    SIG = mybir.ActivationFunctionType.Sigmoid
    MUL, ADD = mybir.AluOpType.mult, mybir.AluOpType.add

    nc = bacc.Bacc("TRN2", target_bir_lowering=False, debug=False)
    tab = nc.dram_tensor("tab", [VOCAB, EMBED], f16, kind="ExternalInput")
    idx = nc.dram_tensor("idx", [128, NT * SCOLS], i16, kind="ExternalInput")
    # pair consts: [s1, s2, k2piv, s4a, s4b, k4piv] each [128, JB, 2]
    kp = nc.dram_tensor("kp", [128, 6 * JB * 2], f16, kind="ExternalInput")
    # mult/add slice offsets depend on host pivot choice -> pass via const
    # tensor is not possible for slicing; host guarantees pivot choice at
    # build time through module-level PIV values set before compile.
    diags = nc.dram_tensor("diags", [128, NDIAG * 128], f16,
                           kind="ExternalInput")
    out = nc.dram_tensor("out", [128, NT * JB * BT], f16,
                         kind="ExternalOutput")

    p1, p2, p4 = _prog_cache["pivots"]
    B = BT
    # slice starts: pivot tap contributes unscaled (the "add" slice);
    # the other tap gets the ratio mult. For a 2-tap conv, tap t uses
    # input positions [t*B : t*B + L*B].
    m1, a1 = (B, 0) if p1 == 0 else (0, B)       # mult-start, add-start
    m2, a2s = (B, 0) if p2 == 0 else (0, B)
    t4a, t4b = [t for t in range(3) if t != p4]  # non-pivot taps of conv4
    PE4 = 5                                       # L4 blocks on PE; rest DVE

    with tile.TileContext(nc) as tc:
        with tc.tile_pool(name="const", bufs=1) as cpool, \
             tc.tile_pool(name="xpa", bufs=2) as xpa, \
             tc.tile_pool(name="xpb", bufs=2) as xpb, \
             tc.tile_pool(name="az", bufs=1) as azp, \
             tc.tile_pool(name="y1", bufs=1) as y1p, \
             tc.tile_pool(name="m2", bufs=1) as m2p, \
             tc.tile_pool(name="y2", bufs=2) as y2p, \
             tc.tile_pool(name="y3", bufs=1) as y3p, \
             tc.tile_pool(name="c4", bufs=2) as c4p, \
             tc.tile_pool(name="y4", bufs=1) as y4p, \
             tc.tile_pool(name="ps", bufs=2, space="PSUM") as pspool:

            idx_sb = cpool.tile([128, NT * SCOLS], i16)
            nc.sync.dma_start(idx_sb[:], idx.ap())
            kp_sb = cpool.tile([128, 6, JB, 2], f16)
            nc.sync.dma_start(kp_sb[:], kp.ap())
            d_sb = cpool.tile([128, NDIAG * 128], f16)
            nc.sync.dma_start(d_sb[:], diags.ap())

            def dg(layer, j, ti):          # layer 0 = L3, 1 = L4
                s = layer * JB * 3 + j * 3 + ti
                return d_sb[:, s * 128:(s + 1) * 128]

            def pair(ap3):
                return ap3.rearrange("p j (a b) -> p j a b", b=2)

            def kb(which, j0, j1, n):
                return kp_sb[:, which, j0:j1].unsqueeze(2).broadcast_to(
                    [128, j1 - j0, n, 2])

            def pe_chain(ps, off, taps, fd):
                segs = []
                c = off
                while c < off + fd:
                    c1 = min((c // 512 + 1) * 512, off + fd)
                    segs.append((c, c1))
                    c = c1
                for ti, (dap, src) in enumerate(taps):
                    for c0, c1 in segs:
                        nc.tensor.matmul(
                            ps[:, c0:c1], dap, src[:, c0 - off:c1 - off],
                            start=(ti == 0), stop=(ti == len(taps) - 1))

            NA, NB = 4 * BT, 3 * BT            # pos 0-3, pos 4-6 rows
            SCA = NA // 16

            def gather(t, xa, xb):
                c0 = t * SCOLS
                nc.gpsimd.dma_gather(
                    xa[:], tab.ap(), idx_sb[:, c0:c0 + SCA],
                    NA, NA, EMBED, transpose=True, single_packet=False)
                nc.gpsimd.dma_gather(
                    xb[:], tab.ap(), idx_sb[:, c0 + SCA:c0 + SCOLS],
                    NB, NB, EMBED, transpose=True, single_packet=False)

            xa0 = xpa.tile([128, JB, NA], f16, tag="xa")
            xb0 = xpb.tile([128, JB, NB], f16, tag="xb")
            xtiles = {0: (xa0, xb0)}
            gather(0, *xtiles[0])

            for t in range(NT):
                if t + 1 < NT:
                    xan = xpa.tile([128, JB, NA], f16, tag="xa")
                    xbn = xpb.tile([128, JB, NB], f16, tag="xb")
                    xtiles[t + 1] = (xan, xbn)
                    gather(t + 1, *xtiles[t + 1])
                xta, xtb = xtiles.pop(t)

                H = JB // 2
                # ---- L1 (half-waves) ----
                az = azp.tile([128, JB, 6 * B], f16, tag="az")
                y1 = y1p.tile([128, JB, 6 * B], f16, tag="y1")
                for j0 in (0, H):
                    j1 = j0 + H
                    mA = NA - m1                 # mult cols from xta
                    nc.vector.tensor_tensor(
                        pair(az[:, j0:j1, 0:mA]),
                        pair(xta[:, j0:j1, m1:NA]),
                        kb(0, j0, j1, mA // 2), MUL)
                    nc.vector.tensor_tensor(
                        pair(az[:, j0:j1, mA:6 * B]),
                        pair(xtb[:, j0:j1, 0:6 * B - mA]),
                        kb(0, j0, j1, (6 * B - mA) // 2), MUL)
                    aA = NA - a1                 # add cols from xta
                    nc.vector.tensor_tensor(
                        az[:, j0:j1, 0:aA], az[:, j0:j1, 0:aA],
                        xta[:, j0:j1, a1:NA], ADD)
                    nc.vector.tensor_tensor(
                        az[:, j0:j1, aA:6 * B], az[:, j0:j1, aA:6 * B],
                        xtb[:, j0:j1, 0:6 * B - aA], ADD)
                    nc.scalar.activation(y1[:, j0:j1, :], az[:, j0:j1, :],
                                         SIG)

                # ---- L2 (half-waves) ----
                m = m2p.tile([128, JB, 5 * B], f16, tag="m2")
                y2 = y2p.tile([128, JB, 5 * B], f16, tag="y2")
                for j0 in (0, H):
                    j1 = j0 + H
                    nc.vector.tensor_tensor(
                        pair(m[:, j0:j1, :]),
                        pair(y1[:, j0:j1, m2:m2 + 5 * B]),
                        kb(1, j0, j1, 5 * B // 2), MUL)
                    nc.vector.tensor_tensor(
                        m[:, j0:j1, :], m[:, j0:j1, :],
                        y1[:, j0:j1, a2s:a2s + 5 * B], ADD)
                    nc.vector.tensor_tensor(
                        pair(m[:, j0:j1, :]), pair(m[:, j0:j1, :]),
                        kb(2, j0, j1, 5 * B // 2), MUL)
                    nc.scalar.activation(y2[:, j0:j1, :], m[:, j0:j1, :],
                                         SIG)

                # ---- L3: PE groups of 2 blocks ----
                y3 = y3p.tile([128, JB, 3 * B], f16, tag="y3")
                for g0 in range(0, JB, 2):
                    ps = pspool.tile([128, 2 * 3 * B], f32, tag="ps")
                    for gi, j in enumerate((g0, g0 + 1)):
                        pe_chain(ps, gi * 3 * B,
                                 [(dg(0, j, ti),
                                   y2[:, j, ti * B:ti * B + 3 * B])
                                  for ti in range(3)], 3 * B)
                    nc.scalar.activation(y3[:, g0:g0 + 2, :], ps[:], SIG)

                # ---- L4: PE blocks 0..PE4-1, DVE ratio-tts for the rest ----
                ps4 = pspool.tile([128, PE4 * B], f32, tag="ps")
                for j in range(PE4):
                    pe_chain(ps4, j * B,
                             [(dg(1, j, ti), y3[:, j, ti * B:(ti + 1) * B])
                              for ti in range(3)], B)
                y4 = y4p.tile([128, JB, B], f16, tag="y4")
                nc.scalar.activation(y4[:, 0:PE4, :], ps4[:], SIG)
                nd = JB - PE4
                c4 = c4p.tile([128, nd, B], f16, tag="c4")
                nc.vector.tensor_tensor(
                    pair(c4[:]),
                    pair(y3[:, PE4:JB, t4a * B:(t4a + 1) * B]),
                    kb(3, PE4, JB, B // 2), MUL)
                nc.vector.tensor_tensor(
                    c4[:], c4[:], y3[:, PE4:JB, p4 * B:(p4 + 1) * B], ADD)
                d4 = c4p.tile([128, nd, B], f16, tag="c4")
                nc.vector.tensor_tensor(
                    pair(d4[:]),
                    pair(y3[:, PE4:JB, t4b * B:(t4b + 1) * B]),
                    kb(4, PE4, JB, B // 2), MUL)
                nc.vector.tensor_tensor(c4[:], c4[:], d4[:], ADD)
                nc.vector.tensor_tensor(
                    pair(c4[:]), pair(c4[:]), kb(5, PE4, JB, B // 2), MUL)
                nc.scalar.activation(y4[:, PE4:JB, :], c4[:], SIG)

                half = JB // 2
                nc.sync.dma_start(
                    out.ap()[:, t * JB * B:t * JB * B + half * B],
                    y4[:, 0:half, :])
                nc.sync.dma_start(
                    out.ap()[:, t * JB * B + half * B:(t + 1) * JB * B],
                    y4[:, half:JB, :])

    nc.compile()
    _prog_cache["nc"] = nc
    return nc


def _pivot(k):
    k = np.asarray(k, np.float64)
    best, bp = None, 0
    for p in range(k.shape[0]):
        mx = np.abs(k / k[p:p + 1]).max()
        if best is None or mx < best:
            best, bp = mx, p
    return bp


def _pack_pairs(conv1, conv2, conv4, p1, p2, p4):
    c1 = np.asarray(conv1, np.float32)
    c2 = np.asarray(conv2, np.float32)
    c4 = np.asarray(conv4, np.float32)
    s1 = c1[1 - p1] / c1[p1]
    s2 = c2[1 - p2] / c2[p2]
    t4a, t4b = [t for t in range(3) if t != p4]
    vecs = (s1, s2, c2[p2], c4[t4a] / c4[p4], c4[t4b] / c4[p4], c4[p4])
    k = np.zeros((128, 6, JB, 2), np.float16)
    for w, vec in enumerate(vecs):
        v = vec.reshape(JB, 128).T
        k[:, w, :, 0] = v
        k[:, w, :, 1] = v
    return k.reshape(128, 6 * JB * 2)


def _pack_diags(conv3, conv4):
    c3 = np.asarray(conv3, np.float32).reshape(3, JB, 128)
    c4 = np.asarray(conv4, np.float32).reshape(3, JB, 128)
    d = np.zeros((128, NDIAG * 128), np.float32)
    s = 0
    for c in (c3, c4):
        for j in range(JB):
            for ti in range(3):
                np.fill_diagonal(d[:, s * 128:(s + 1) * 128], c[ti, j])
                s += 1
    return d.astype(np.float16)


def _make_idx(Xc):
    o = np.zeros((128, NT * SCOLS), np.int16)
    for t in range(NT):
        rows = Xc[t * BT:(t + 1) * BT, :]
        for g, sl in ((0, slice(0, 4)), (1, slice(4, 7))):
            flat = rows[:, sl].T.reshape(-1)
            sc = len(flat) // 16
            wrap = flat.reshape(sc, 16).T.astype(np.int16)
            c0 = t * SCOLS + (0 if g == 0 else 4 * BT // 16)
            for m in range(8):
                o[16 * m:16 * m + 16, c0:c0 + sc] = wrap
    return o


def _unpermute(raw):
    a = np.asarray(raw, np.float32).reshape(128, NT, JB, BT)
    return np.ascontiguousarray(
        a.transpose(1, 3, 2, 0).reshape(BCORE, EMBED))


def run(X, emb, conv1, conv2, conv3, conv4, **spmd_kwargs):
    X = np.asarray(X)
    emb = np.asarray(emb, np.float32)
    c1 = np.asarray(conv1, np.float32)
    p1, p2, p4 = _pivot(conv1), _pivot(conv2), _pivot(conv4)
    _prog_cache.setdefault("pivots", (p1, p2, p4))
    nc = _build_program()

    table = (emb * c1[p1][None, :]).astype(np.float16)
    kpack = _pack_pairs(conv1, conv2, conv4, p1, p2, p4)
    dpack = _pack_diags(conv3, conv4)

    in_maps = []
    for c in range(NCORES):
        Xc = X[c * BCORE:(c + 1) * BCORE]
        in_maps.append({"tab": table, "idx": _make_idx(Xc), "kp": kpack,
                        "diags": dpack})

    res = bass_utils.run_bass_kernel_spmd(nc, in_maps,
                                          core_ids=list(range(NCORES)),
                                          **spmd_kwargs)
    o = np.concatenate(
        [_unpermute(res.results[c]["out"]) for c in range(NCORES)], axis=0)
    return o, res


def kernel(X, emb, conv1, conv2, conv3, conv4):
    o, _ = run(X, emb, conv1, conv2, conv3, conv4)
    return o


# revision 3
# speedup vs baseline: 1.0388x; 1.0388x over previous
"""v4 Trainium2 Bass kernel for nn_CSM_62216896250023 (dense_cnn).

One merged sigmoid ACT instruction per layer per tile (7-8 ACT/tile).
Single SWDGE gather per tile from a host-prescaled table (emb * pivot
tap of conv1). Conv taps applied with pair-broadcast tensor_tensor
(2x DVE fast mode, per-channel constants as packed [.,2] pairs):
  L1: a = x'[mult]*S1 (tt), Z1 = a += x'[add] (tt, in-place)
  L2: m = Y1[mult]*S2, m += Y1[add], Z2 = m *= Kpivot (DVE j0-3,
      Pool j4-7)
  L3: PE true-diag 3-tap chains, psum groups {2,2,2,2}
  L4: PE true-diag chains -> one psum group
All fp16; output fp16 -> host fp32.
"""
import numpy as np

import concourse.bacc as bacc
import concourse.tile as tile
import concourse.bass_utils as bass_utils
from concourse import mybir

VOCAB, EMBED, BATCH, SEQ = 32000, 1024, 16384, 7
NCORES = 8
BCORE = BATCH // NCORES          # 2048
BT = 256
NT = BCORE // BT                 # 8 tiles
JB = EMBED // 128                # 8
NIDX = SEQ * BT                  # 1792
SCOLS = NIDX // 16               # 112
NDIAG = JB * 3 * 2               # 48 (L3 + L4)
POOL_KLAST_J0 = 4                # j4-7 L2 k-pivot mult on Pool

_prog_cache = {}


def _build_program():
    if "nc" in _prog_cache:
        return _prog_cache["nc"]
    f32, f16, i16 = mybir.dt.float32, mybir.dt.float16, mybir.dt.int16
    SIG = mybir.ActivationFunctionType.Sigmoid
    MUL, ADD = mybir.AluOpType.mult, mybir.AluOpType.add

    nc = bacc.Bacc("TRN2", target_bir_lowering=False, debug=False)
    tab = nc.dram_tensor("tab", [VOCAB, EMBED], f16, kind="ExternalInput")
    idx = nc.dram_tensor("idx", [128, NT * SCOLS], i16, kind="ExternalInput")
    # pair consts: [s1, s2, k2piv, s4a, s4b, k4piv] each [128, JB, 2]
    kp = nc.dram_tensor("kp", [128, 6 * JB * 2], f16, kind="ExternalInput")
    # mult/add slice offsets depend on host pivot choice -> pass via const
    # tensor is not possible for slicing; host guarantees pivot choice at
    # build time through module-level PIV values set before compile.
    diags = nc.dram_tensor("diags", [128, NDIAG * 128], f16,
                           kind="ExternalInput")
    out = nc.dram_tensor("out", [128, NT * JB * BT], f16,
                         kind="ExternalOutput")

    p1, p2, p4 = _prog_cache["pivots"]
    B = BT
    # slice starts: pivot tap contributes unscaled (the "add" slice);
    # the other tap gets the ratio mult. For a 2-tap conv, tap t uses
    # input positions [t*B : t*B + L*B].
    m1, a1 = (B, 0) if p1 == 0 else (0, B)       # mult-start, add-start
    m2, a2s = (B, 0) if p2 == 0 else (0, B)
    t4a, t4b = [t for t in range(3) if t != p4]  # non-pivot taps of conv4
    PE4 = 5                                       # L4 blocks on PE; rest DVE

    with tile.TileContext(nc) as tc:
        with tc.tile_pool(name="const", bufs=1) as cpool, \
             tc.tile_pool(name="xpa", bufs=2) as xpa, \
             tc.tile_pool(name="xpb", bufs=2) as xpb, \
             tc.tile_pool(name="az", bufs=1) as azp, \
             tc.tile_pool(name="y1", bufs=1) as y1p, \
             tc.tile_pool(name="m2", bufs=1) as m2p, \
             tc.tile_pool(name="y2", bufs=2) as y2p, \
             tc.tile_pool(name="y3", bufs=1) as y3p, \
             tc.tile_pool(name="c4", bufs=2) as c4p, \
             tc.tile_pool(name="y4", bufs=1) as y4p, \
             tc.tile_pool(name="ps", bufs=2, space="PSUM") as pspool:

            idx_sb = cpool.tile([128, NT * SCOLS], i16)
            nc.sync.dma_start(idx_sb[:], idx.ap())
            kp_sb = cpool.tile([128, 6, JB, 2], f16)
            nc.sync.dma_start(kp_sb[:], kp.ap())
            d_sb = cpool.tile([128, NDIAG * 128], f16)
            nc.sync.dma_start(d_sb[:], diags.ap())

            def dg(layer, j, ti):          # layer 0 = L3, 1 = L4
                s = layer * JB * 3 + j * 3 + ti
                return d_sb[:, s * 128:(s + 1) * 128]

            def pair(ap3):
                return ap3.rearrange("p j (a b) -> p j a b", b=2)

            def kb(which, j0, j1, n):
                return kp_sb[:, which, j0:j1].unsqueeze(2).broadcast_to(
                    [128, j1 - j0, n, 2])

            def pe_chain(ps, off, taps, fd):
                segs = []
                c = off
                while c < off + fd:
                    c1 = min((c // 512 + 1) * 512, off + fd)
                    segs.append((c, c1))
                    c = c1
                for ti, (dap, src) in enumerate(taps):
                    for c0, c1 in segs:
                        nc.tensor.matmul(
                            ps[:, c0:c1], dap, src[:, c0 - off:c1 - off],
                            start=(ti == 0), stop=(ti == len(taps) - 1))

            NA, NB = 4 * BT, 3 * BT            # pos 0-3, pos 4-6 rows
            SCA = NA // 16

            def gather(t, xa, xb):
                c0 = t * SCOLS
                nc.gpsimd.dma_gather(
                    xa[:], tab.ap(), idx_sb[:, c0:c0 + SCA],
                    NA, NA, EMBED, transpose=True, single_packet=False)
                nc.gpsimd.dma_gather(
                    xb[:], tab.ap(), idx_sb[:, c0 + SCA:c0 + SCOLS],
                    NB, NB, EMBED, transpose=True, single_packet=False)

            xa0 = xpa.tile([128, JB, NA], f16, tag="xa")
            xb0 = xpb.tile([128, JB, NB], f16, tag="xb")
            xtiles = {0: (xa0, xb0)}
            gather(0, *xtiles[0])

            for t in range(NT):
                if t + 1 < NT:
                    xan = xpa.tile([128, JB, NA], f16, tag="xa")
                    xbn = xpb.tile([128, JB, NB], f16, tag="xb")
                    xtiles[t + 1] = (xan, xbn)
                    gather(t + 1, *xtiles[t + 1])
                xta, xtb = xtiles.pop(t)

                H = JB // 2
                # ---- L1 (half-waves) ----
                az = azp.tile([128, JB, 6 * B], f16, tag="az")
                y1 = y1p.tile([128, JB, 6 * B], f16, tag="y1")
                for j0 in (0, H):
                    j1 = j0 + H
                    mA = NA - m1                 # mult cols from xta
                    nc.vector.tensor_tensor(
                        pair(az[:, j0:j1, 0:mA]),
                        pair(xta[:, j0:j1, m1:NA]),
                        kb(0, j0, j1, mA // 2), MUL)
                    nc.vector.tensor_tensor(
                        pair(az[:, j0:j1, mA:6 * B]),
                        pair(xtb[:, j0:j1, 0:6 * B - mA]),
                        kb(0, j0, j1, (6 * B - mA) // 2), MUL)
                    aA = NA - a1                 # add cols from xta
                    nc.vector.tensor_tensor(
                        az[:, j0:j1, 0:aA], az[:, j0:j1, 0:aA],
                        xta[:, j0:j1, a1:NA], ADD)
                    nc.vector.tensor_tensor(
                        az[:, j0:j1, aA:6 * B], az[:, j0:j1, aA:6 * B],
                        xtb[:, j0:j1, 0:6 * B - aA], ADD)
                    nc.scalar.activation(y1[:, j0:j1, :], az[:, j0:j1, :],
                                         SIG)

                # ---- L2 (half-waves) ----
                m = m2p.tile([128, JB, 5 * B], f16, tag="m2")
                y2 = y2p.tile([128, JB, 5 * B], f16, tag="y2")
                for j0 in (0, H):
                    j1 = j0 + H
                    nc.vector.tensor_tensor(
                        pair(m[:, j0:j1, :]),
                        pair(y1[:, j0:j1, m2:m2 + 5 * B]),
                        kb(1, j0, j1, 5 * B // 2), MUL)
                    nc.vector.tensor_tensor(
                        m[:, j0:j1, :], m[:, j0:j1, :],
                        y1[:, j0:j1, a2s:a2s + 5 * B], ADD)
                    nc.vector.tensor_tensor(
                        pair(m[:, j0:j1, :]), pair(m[:, j0:j1, :]),
                        kb(2, j0, j1, 5 * B // 2), MUL)
                    nc.scalar.activation(y2[:, j0:j1, :], m[:, j0:j1, :],
                                         SIG)

                # ---- L3: PE groups of 2 blocks ----
                y3 = y3p.tile([128, JB, 3 * B], f16, tag="y3")
                for g0 in range(0, JB, 2):
                    ps = pspool.tile([128, 2 * 3 * B], f32, tag="ps")
                    for gi, j in enumerate((g0, g0 + 1)):
                        pe_chain(ps, gi * 3 * B,
                                 [(dg(0, j, ti),
                                   y2[:, j, ti * B:ti * B + 3 * B])
                                  for ti in range(3)], 3 * B)
                    nc.scalar.activation(y3[:, g0:g0 + 2, :], ps[:], SIG)

                # ---- L4: PE blocks 0..PE4-1, DVE ratio-tts for the rest ----
                ps4 = pspool.tile([128, PE4 * B], f32, tag="ps")
                for j in range(PE4):
                    pe_chain(ps4, j * B,
                             [(dg(1, j, ti), y3[:, j, ti * B:(ti + 1) * B])
                              for ti in range(3)], B)
                y4 = y4p.tile([128, JB, B], f16, tag="y4")
                nc.scalar.activation(y4[:, 0:PE4, :], ps4[:], SIG)
                nd = JB - PE4
                c4 = c4p.tile([128, nd, B], f16, tag="c4")
                nc.vector.tensor_tensor(
                    pair(c4[:]),
                    pair(y3[:, PE4:JB, t4a * B:(t4a + 1) * B]),
                    kb(3, PE4, JB, B // 2), MUL)
                nc.vector.tensor_tensor(
                    c4[:], c4[:], y3[:, PE4:JB, p4 * B:(p4 + 1) * B], ADD)
                d4 = c4p.tile([128, nd, B], f16, tag="c4")
                nc.vector.tensor_tensor(
                    pair(d4[:]),
                    pair(y3[:, PE4:JB, t4b * B:(t4b + 1) * B]),
                    kb(4, PE4, JB, B // 2), MUL)
                nc.vector.tensor_tensor(c4[:], c4[:], d4[:], ADD)
                nc.vector.tensor_tensor(
                    pair(c4[:]), pair(c4[:]), kb(5, PE4, JB, B // 2), MUL)
                nc.scalar.activation(y4[:, PE4:JB, :], c4[:], SIG)

                half = JB // 2
                nc.sync.dma_start(
                    out.ap()[:, t * JB * B:t * JB * B + half * B],
                    y4[:, 0:half, :])
                nc.sync.dma_start(
                    out.ap()[:, t * JB * B + half * B:(t + 1) * JB * B],
                    y4[:, half:JB, :])

    nc.compile()
    _prog_cache["nc"] = nc
    return nc


def _pivot(k):
    k = np.asarray(k, np.float64)
    best, bp = None, 0
    for p in range(k.shape[0]):
        mx = np.abs(k / k[p:p + 1]).max()
        if best is None or mx < best:
            best, bp = mx, p
    return bp


def _pack_pairs(conv1, conv2, conv4, p1, p2, p4):
    c1 = np.asarray(conv1, np.float32)
    c2 = np.asarray(conv2, np.float32)
    c4 = np.asarray(conv4, np.float32)
    s1 = c1[1 - p1] / c1[p1]
    s2 = c2[1 - p2] / c2[p2]
    t4a, t4b = [t for t in range(3) if t != p4]
    vecs = (s1, s2, c2[p2], c4[t4a] / c4[p4], c4[t4b] / c4[p4], c4[p4])
    k = np.zeros((128, 6, JB, 2), np.float16)
    for w, vec in enumerate(vecs):
        v = vec.reshape(JB, 128).T
        k[:, w, :, 0] = v
        k[:, w, :, 1] = v
    return k.reshape(128, 6 * JB * 2)


def _pack_diags(conv3, conv4):
    c3 = np.asarray(conv3, np.float32).reshape(3, JB, 128)
    c4 = np.asarray(conv4, np.float32).reshape(3, JB, 128)
    d = np.zeros((128, NDIAG * 128), np.float32)
    s = 0
    for c in (c3, c4):
        for j in range(JB):
            for ti in range(3):
                np.fill_diagonal(d[:, s * 128:(s + 1) * 128], c[ti, j])
                s += 1
    return d.astype(np.float16)


def _make_idx(Xc):
    o = np.zeros((128, NT * SCOLS), np.int16)
    for t in range(NT):
        rows = Xc[t * BT:(t + 1) * BT, :]
        for g, sl in ((0, slice(0, 4)), (1, slice(4, 7))):
            flat = rows[:, sl].T.reshape(-1)
            sc = len(flat) // 16
            wrap = flat.reshape(sc, 16).T.astype(np.int16)
            c0 = t * SCOLS + (0 if g == 0 else 4 * BT // 16)
            for m in range(8):
                o[16 * m:16 * m + 16, c0:c0 + sc] = wrap
    return o


def _unpermute(raw):
    a = np.asarray(raw, np.float32).reshape(128, NT, JB, BT)
    return np.ascontiguousarray(
        a.transpose(1, 3, 2, 0).reshape(BCORE, EMBED))


def run(X, emb, conv1, conv2, conv3, conv4, **spmd_kwargs):
    X = np.asarray(X)
    emb = np.asarray(emb, np.float32)
    c1 = np.asarray(conv1, np.float32)
    p1, p2, p4 = _prog_cache.setdefault(
        "pivots", (_pivot(conv1), _pivot(conv2), _pivot(conv4)))
    nc = _build_program()

    table = (emb * c1[p1][None, :]).astype(np.float16)
    kpack = _pack_pairs(conv1, conv2, conv4, p1, p2, p4)
    dpack = _pack_diags(conv3, conv4)

    in_maps = []
    for c in range(NCORES):
        Xc = X[c * BCORE:(c + 1) * BCORE]
        in_maps.append({"tab": table, "idx": _make_idx(Xc), "kp": kpack,
                        "diags": dpack})

    res = bass_utils.run_bass_kernel_spmd(nc, in_maps,
                                          core_ids=list(range(NCORES)),
                                          **spmd_kwargs)
    o = np.concatenate(
        [_unpermute(res.results[c]["out"]) for c in range(NCORES)], axis=0)
    return o, res


def kernel(X, emb, conv1, conv2, conv3, conv4):
    o, _ = run(X, emb, conv1, conv2, conv3, conv4)
    return o


# revision 4
# speedup vs baseline: 1.0428x; 1.0039x over previous
"""v4 Trainium2 Bass kernel for nn_CSM_62216896250023 (dense_cnn).

One merged sigmoid ACT instruction per layer per tile (7-8 ACT/tile).
Single SWDGE gather per tile from a host-prescaled table (emb * pivot
tap of conv1). Conv taps applied with pair-broadcast tensor_tensor
(2x DVE fast mode, per-channel constants as packed [.,2] pairs):
  L1: a = x'[mult]*S1 (tt), Z1 = a += x'[add] (tt, in-place)
  L2: m = Y1[mult]*S2, m += Y1[add], Z2 = m *= Kpivot (DVE j0-3,
      Pool j4-7)
  L3: PE true-diag 3-tap chains, psum groups {2,2,2,2}
  L4: PE true-diag chains -> one psum group
All fp16; output fp16 -> host fp32.
"""
import numpy as np

import concourse.bacc as bacc
import concourse.tile as tile
import concourse.bass_utils as bass_utils
from concourse import mybir

VOCAB, EMBED, BATCH, SEQ = 32000, 1024, 16384, 7
NCORES = 8
BCORE = BATCH // NCORES          # 2048
BT = 256
NT = BCORE // BT                 # 8 tiles
# ramp/drain compression: small tiles first and last so the pipeline
# fills and drains quickly; gather rows must stay multiples of 128
SCHED = (128, 128, 256, 256, 256, 256, 256, 256, 128, 128)
assert sum(SCHED) == BCORE
JB = EMBED // 128                # 8
NIDX = SEQ * BT                  # 1792
SCOLS = NIDX // 16               # 112
NDIAG = JB * 3 * 2               # 48 (L3 + L4)
POOL_KLAST_J0 = 4                # j4-7 L2 k-pivot mult on Pool

_prog_cache = {}


def _build_program():
    if "nc" in _prog_cache:
        return _prog_cache["nc"]
    f32, f16, i16 = mybir.dt.float32, mybir.dt.float16, mybir.dt.int16
    SIG = mybir.ActivationFunctionType.Sigmoid
    MUL, ADD = mybir.AluOpType.mult, mybir.AluOpType.add

    nc = bacc.Bacc("TRN2", target_bir_lowering=False, debug=False)
    tab = nc.dram_tensor("tab", [VOCAB, EMBED], f16, kind="ExternalInput")
    idx = nc.dram_tensor("idx", [128, NT * SCOLS], i16, kind="ExternalInput")
    # pair consts: [s1, s2, k2piv, s4a, s4b, k4piv] each [128, JB, 2]
    kp = nc.dram_tensor("kp", [128, 6 * JB * 2], f16, kind="ExternalInput")
    # mult/add slice offsets depend on host pivot choice -> pass via const
    # tensor is not possible for slicing; host guarantees pivot choice at
    # build time through module-level PIV values set before compile.
    diags = nc.dram_tensor("diags", [128, NDIAG * 128], f16,
                           kind="ExternalInput")
    out = nc.dram_tensor("out", [128, NT * JB * BT], f16,
                         kind="ExternalOutput")

    p1, p2, p4 = _prog_cache["pivots"]
    t4a, t4b = [t for t in range(3) if t != p4]  # non-pivot taps of conv4
    PE4 = 5                                       # L4 blocks on PE; rest DVE

    with tile.TileContext(nc) as tc:
        with tc.tile_pool(name="const", bufs=1) as cpool, \
             tc.tile_pool(name="xpa", bufs=2) as xpa, \
             tc.tile_pool(name="xpb", bufs=2) as xpb, \
             tc.tile_pool(name="az", bufs=1) as azp, \
             tc.tile_pool(name="y1", bufs=1) as y1p, \
             tc.tile_pool(name="m2", bufs=1) as m2p, \
             tc.tile_pool(name="y2", bufs=2) as y2p, \
             tc.tile_pool(name="y3", bufs=1) as y3p, \
             tc.tile_pool(name="c4", bufs=2) as c4p, \
             tc.tile_pool(name="y4", bufs=1) as y4p, \
             tc.tile_pool(name="ps", bufs=2, space="PSUM") as pspool:

            idx_sb = cpool.tile([128, NT * SCOLS], i16)
            nc.sync.dma_start(idx_sb[:], idx.ap())
            kp_sb = cpool.tile([128, 6, JB, 2], f16)
            nc.sync.dma_start(kp_sb[:], kp.ap())
            d_sb = cpool.tile([128, NDIAG * 128], f16)
            nc.sync.dma_start(d_sb[:], diags.ap())

            def dg(layer, j, ti):          # layer 0 = L3, 1 = L4
                s = layer * JB * 3 + j * 3 + ti
                return d_sb[:, s * 128:(s + 1) * 128]

            def pair(ap3):
                return ap3.rearrange("p j (a b) -> p j a b", b=2)

            def kb(which, j0, j1, n):
                return kp_sb[:, which, j0:j1].unsqueeze(2).broadcast_to(
                    [128, j1 - j0, n, 2])

            def pe_chain(ps, off, taps, fd):
                segs = []
                c = off
                while c < off + fd:
                    c1 = min((c // 512 + 1) * 512, off + fd)
                    segs.append((c, c1))
                    c = c1
                for ti, (dap, src) in enumerate(taps):
                    for c0, c1 in segs:
                        nc.tensor.matmul(
                            ps[:, c0:c1], dap, src[:, c0 - off:c1 - off],
                            start=(ti == 0), stop=(ti == len(taps) - 1))

            NTT = len(SCHED)
            ioff = [0]
            roff = [0]
            for bt in SCHED:
                ioff.append(ioff[-1] + 7 * bt // 16)
                roff.append(roff[-1] + bt)

            def gather(t, xa, xb):
                bt = SCHED[t]
                na, nb = 4 * bt, 3 * bt
                c0 = ioff[t]
                sca = na // 16
                nc.gpsimd.dma_gather(
                    xa[:], tab.ap(), idx_sb[:, c0:c0 + sca],
                    na, na, EMBED, transpose=True, single_packet=False)
                nc.gpsimd.dma_gather(
                    xb[:], tab.ap(), idx_sb[:, c0 + sca:ioff[t + 1]],
                    nb, nb, EMBED, transpose=True, single_packet=False)

            xa0 = xpa.tile([128, JB, 4 * SCHED[0]], f16, tag="xa")
            xb0 = xpb.tile([128, JB, 3 * SCHED[0]], f16, tag="xb")
            xtiles = {0: (xa0, xb0)}
            gather(0, *xtiles[0])

            for t in range(NTT):
                B = SCHED[t]
                NA = 4 * B
                m1, a1 = (B, 0) if p1 == 0 else (0, B)
                m2, a2s = (B, 0) if p2 == 0 else (0, B)
                if t + 1 < NTT:
                    xan = xpa.tile([128, JB, 4 * SCHED[t + 1]], f16,
                                   tag="xa")
                    xbn = xpb.tile([128, JB, 3 * SCHED[t + 1]], f16,
                                   tag="xb")
                    xtiles[t + 1] = (xan, xbn)
                    gather(t + 1, *xtiles[t + 1])
                xta, xtb = xtiles.pop(t)

                H = JB // 2
                # ---- L1 (half-waves) ----
                az = azp.tile([128, JB, 6 * B], f16, tag="az")
                y1 = y1p.tile([128, JB, 6 * B], f16, tag="y1")
                for j0 in (0, H):
                    j1 = j0 + H
                    mA = NA - m1                 # mult cols from xta
                    nc.vector.tensor_tensor(
                        pair(az[:, j0:j1, 0:mA]),
                        pair(xta[:, j0:j1, m1:NA]),
                        kb(0, j0, j1, mA // 2), MUL)
                    nc.vector.tensor_tensor(
                        pair(az[:, j0:j1, mA:6 * B]),
                        pair(xtb[:, j0:j1, 0:6 * B - mA]),
                        kb(0, j0, j1, (6 * B - mA) // 2), MUL)
                    aA = NA - a1                 # add cols from xta
                    nc.vector.tensor_tensor(
                        az[:, j0:j1, 0:aA], az[:, j0:j1, 0:aA],
                        xta[:, j0:j1, a1:NA], ADD)
                    nc.vector.tensor_tensor(
                        az[:, j0:j1, aA:6 * B], az[:, j0:j1, aA:6 * B],
                        xtb[:, j0:j1, 0:6 * B - aA], ADD)
                    nc.scalar.activation(y1[:, j0:j1, :], az[:, j0:j1, :],
                                         SIG)

                # ---- L2 (half-waves) ----
                m = m2p.tile([128, JB, 5 * B], f16, tag="m2")
                y2 = y2p.tile([128, JB, 5 * B], f16, tag="y2")
                for j0 in (0, H):
                    j1 = j0 + H
                    nc.vector.tensor_tensor(
                        pair(m[:, j0:j1, :]),
                        pair(y1[:, j0:j1, m2:m2 + 5 * B]),
                        kb(1, j0, j1, 5 * B // 2), MUL)
                    nc.vector.tensor_tensor(
                        m[:, j0:j1, :], m[:, j0:j1, :],
                        y1[:, j0:j1, a2s:a2s + 5 * B], ADD)
                    nc.vector.tensor_tensor(
                        pair(m[:, j0:j1, :]), pair(m[:, j0:j1, :]),
                        kb(2, j0, j1, 5 * B // 2), MUL)
                    nc.scalar.activation(y2[:, j0:j1, :], m[:, j0:j1, :],
                                         SIG)

                # ---- L3: PE groups of 2 blocks ----
                y3 = y3p.tile([128, JB, 3 * B], f16, tag="y3")
                for g0 in range(0, JB, 2):
                    ps = pspool.tile([128, 2 * 3 * B], f32, tag="ps")
                    for gi, j in enumerate((g0, g0 + 1)):
                        pe_chain(ps, gi * 3 * B,
                                 [(dg(0, j, ti),
                                   y2[:, j, ti * B:ti * B + 3 * B])
                                  for ti in range(3)], 3 * B)
                    nc.scalar.activation(y3[:, g0:g0 + 2, :], ps[:], SIG)

                # ---- L4: PE blocks 0..PE4-1, DVE ratio-tts for the rest ----
                ps4 = pspool.tile([128, PE4 * B], f32, tag="ps")
                for j in range(PE4):
                    pe_chain(ps4, j * B,
                             [(dg(1, j, ti), y3[:, j, ti * B:(ti + 1) * B])
                              for ti in range(3)], B)
                y4 = y4p.tile([128, JB, B], f16, tag="y4")
                nc.scalar.activation(y4[:, 0:PE4, :], ps4[:], SIG)
                nd = JB - PE4
                c4 = c4p.tile([128, nd, B], f16, tag="c4")
                nc.vector.tensor_tensor(
                    pair(c4[:]),
                    pair(y3[:, PE4:JB, t4a * B:(t4a + 1) * B]),
                    kb(3, PE4, JB, B // 2), MUL)
                nc.vector.tensor_tensor(
                    c4[:], c4[:], y3[:, PE4:JB, p4 * B:(p4 + 1) * B], ADD)
                d4 = c4p.tile([128, nd, B], f16, tag="c4")
                nc.vector.tensor_tensor(
                    pair(d4[:]),
                    pair(y3[:, PE4:JB, t4b * B:(t4b + 1) * B]),
                    kb(4, PE4, JB, B // 2), MUL)
                nc.vector.tensor_tensor(c4[:], c4[:], d4[:], ADD)
                nc.vector.tensor_tensor(
                    pair(c4[:]), pair(c4[:]), kb(5, PE4, JB, B // 2), MUL)
                nc.scalar.activation(y4[:, PE4:JB, :], c4[:], SIG)

                half = JB // 2
                ob = JB * roff[t]
                nc.sync.dma_start(
                    out.ap()[:, ob:ob + half * B], y4[:, 0:half, :])
                nc.sync.dma_start(
                    out.ap()[:, ob + half * B:ob + JB * B],
                    y4[:, half:JB, :])

    nc.compile()
    _prog_cache["nc"] = nc
    return nc


def _pivot(k):
    k = np.asarray(k, np.float64)
    best, bp = None, 0
    for p in range(k.shape[0]):
        mx = np.abs(k / k[p:p + 1]).max()
        if best is None or mx < best:
            best, bp = mx, p
    return bp


def _pack_pairs(conv1, conv2, conv4, p1, p2, p4):
    c1 = np.asarray(conv1, np.float32)
    c2 = np.asarray(conv2, np.float32)
    c4 = np.asarray(conv4, np.float32)
    s1 = c1[1 - p1] / c1[p1]
    s2 = c2[1 - p2] / c2[p2]
    t4a, t4b = [t for t in range(3) if t != p4]
    vecs = (s1, s2, c2[p2], c4[t4a] / c4[p4], c4[t4b] / c4[p4], c4[p4])
    k = np.zeros((128, 6, JB, 2), np.float16)
    for w, vec in enumerate(vecs):
        v = vec.reshape(JB, 128).T
        k[:, w, :, 0] = v
        k[:, w, :, 1] = v
    return k.reshape(128, 6 * JB * 2)


def _pack_diags(conv3, conv4):
    c3 = np.asarray(conv3, np.float32).reshape(3, JB, 128)
    c4 = np.asarray(conv4, np.float32).reshape(3, JB, 128)
    d = np.zeros((128, NDIAG * 128), np.float32)
    s = 0
    for c in (c3, c4):
        for j in range(JB):
            for ti in range(3):
                np.fill_diagonal(d[:, s * 128:(s + 1) * 128], c[ti, j])
                s += 1
    return d.astype(np.float16)


def _make_idx(Xc):
    o = np.zeros((128, NT * SCOLS), np.int16)
    r0, c0 = 0, 0
    for bt in SCHED:
        rows = Xc[r0:r0 + bt, :]
        for sl in (slice(0, 4), slice(4, 7)):
            flat = rows[:, sl].T.reshape(-1)
            sc = len(flat) // 16
            wrap = flat.reshape(sc, 16).T.astype(np.int16)
            for m in range(8):
                o[16 * m:16 * m + 16, c0:c0 + sc] = wrap
            c0 += sc
        r0 += bt
    return o


def _unpermute(raw):
    raw = np.asarray(raw, np.float32)
    o = np.empty((BCORE, EMBED), np.float32)
    r0 = 0
    for bt in SCHED:
        a = raw[:, JB * r0:JB * (r0 + bt)].reshape(128, JB, bt)
        o[r0:r0 + bt] = a.transpose(2, 1, 0).reshape(bt, EMBED)
        r0 += bt
    return o


def run(X, emb, conv1, conv2, conv3, conv4, **spmd_kwargs):
    X = np.asarray(X)
    emb = np.asarray(emb, np.float32)
    c1 = np.asarray(conv1, np.float32)
    p1, p2, p4 = _prog_cache.setdefault(
        "pivots", (_pivot(conv1), _pivot(conv2), _pivot(conv4)))
    nc = _build_program()

    table = (emb * c1[p1][None, :]).astype(np.float16)
    kpack = _pack_pairs(conv1, conv2, conv4, p1, p2, p4)
    dpack = _pack_diags(conv3, conv4)

    in_maps = []
    for c in range(NCORES):
        Xc = X[c * BCORE:(c + 1) * BCORE]
        in_maps.append({"tab": table, "idx": _make_idx(Xc), "kp": kpack,
                        "diags": dpack})

    res = bass_utils.run_bass_kernel_spmd(nc, in_maps,
                                          core_ids=list(range(NCORES)),
                                          **spmd_kwargs)
    o = np.concatenate(
        [_unpermute(res.results[c]["out"]) for c in range(NCORES)], axis=0)
    return o, res


def kernel(X, emb, conv1, conv2, conv3, conv4):
    o, _ = run(X, emb, conv1, conv2, conv3, conv4)
    return o


# revision 5
# speedup vs baseline: 1.0699x; 1.0260x over previous
"""v4 Trainium2 Bass kernel for nn_CSM_62216896250023 (dense_cnn).

One merged sigmoid ACT instruction per layer per tile (7-8 ACT/tile).
Single SWDGE gather per tile from a host-prescaled table (emb * pivot
tap of conv1). Conv taps applied with pair-broadcast tensor_tensor
(2x DVE fast mode, per-channel constants as packed [.,2] pairs):
  L1: a = x'[mult]*S1 (tt), Z1 = a += x'[add] (tt, in-place)
  L2: m = Y1[mult]*S2, m += Y1[add], Z2 = m *= Kpivot (DVE j0-3,
      Pool j4-7)
  L3: PE true-diag 3-tap chains, psum groups {2,2,2,2}
  L4: PE true-diag chains -> one psum group
All fp16; output fp16 -> host fp32.
"""
import numpy as np

import concourse.bacc as bacc
import concourse.tile as tile
import concourse.bass_utils as bass_utils
from concourse import mybir

VOCAB, EMBED, BATCH, SEQ = 32000, 1024, 16384, 7
NCORES = 8
BCORE = BATCH // NCORES          # 2048
BT = 256
NT = BCORE // BT                 # 8 tiles
# ramp/drain compression: small tiles first and last so the pipeline
# fills and drains quickly; gather rows must stay multiples of 128
SCHED = (128, 128, 256, 256, 256, 256, 256, 256, 128, 128)
assert sum(SCHED) == BCORE
JB = EMBED // 128                # 8
NIDX = SEQ * BT                  # 1792
SCOLS = NIDX // 16               # 112
NDIAG = JB * 3 * 2               # 48 (L3 + L4)
POOL_KLAST_J0 = 4                # j4-7 L2 k-pivot mult on Pool

_prog_cache = {}


def _build_program():
    if "nc" in _prog_cache:
        return _prog_cache["nc"]
    f32, f16, i16 = mybir.dt.float32, mybir.dt.float16, mybir.dt.int16
    SIG = mybir.ActivationFunctionType.Sigmoid
    MUL, ADD = mybir.AluOpType.mult, mybir.AluOpType.add

    nc = bacc.Bacc("TRN2", target_bir_lowering=False, debug=False)
    tab = nc.dram_tensor("tab", [VOCAB, EMBED], f16, kind="ExternalInput")
    idx = nc.dram_tensor("idx", [128, NT * SCOLS], i16, kind="ExternalInput")
    # pair consts: [s1, s2, k2piv, s4a, s4b, k4piv] each [128, JB, 2]
    kp = nc.dram_tensor("kp", [128, 6 * JB * 2], f16, kind="ExternalInput")
    # mult/add slice offsets depend on host pivot choice -> pass via const
    # tensor is not possible for slicing; host guarantees pivot choice at
    # build time through module-level PIV values set before compile.
    diags = nc.dram_tensor("diags", [128, NDIAG * 128], f16,
                           kind="ExternalInput")
    out = nc.dram_tensor("out", [128, NT * JB * BT], f16,
                         kind="ExternalOutput")

    p1, p2, p4 = _prog_cache["pivots"]
    t4a, t4b = [t for t in range(3) if t != p4]  # non-pivot taps of conv4
    PE4 = 5                                       # L4 blocks on PE; rest DVE

    with tile.TileContext(nc) as tc:
        with tc.tile_pool(name="const", bufs=1) as cpool, \
             tc.tile_pool(name="xpa", bufs=2) as xpa, \
             tc.tile_pool(name="xpb", bufs=2) as xpb, \
             tc.tile_pool(name="az", bufs=1) as azp, \
             tc.tile_pool(name="y1", bufs=1) as y1p, \
             tc.tile_pool(name="m2", bufs=1) as m2p, \
             tc.tile_pool(name="y2", bufs=1) as y2p, \
             tc.tile_pool(name="y3", bufs=2) as y3p, \
             tc.tile_pool(name="c4", bufs=2) as c4p, \
             tc.tile_pool(name="y4", bufs=1) as y4p, \
             tc.tile_pool(name="ps", bufs=2, space="PSUM") as pspool:

            idx_sb = cpool.tile([128, NT * SCOLS], i16)
            nc.sync.dma_start(idx_sb[:], idx.ap())
            kp_sb = cpool.tile([128, 6, JB, 2], f16)
            nc.sync.dma_start(kp_sb[:], kp.ap())
            d_sb = cpool.tile([128, NDIAG * 128], f16)
            nc.sync.dma_start(d_sb[:], diags.ap())

            def dg(layer, j, ti):          # layer 0 = L3, 1 = L4
                s = layer * JB * 3 + j * 3 + ti
                return d_sb[:, s * 128:(s + 1) * 128]

            def pair(ap3):
                return ap3.rearrange("p j (a b) -> p j a b", b=2)

            def kb(which, j0, j1, n):
                return kp_sb[:, which, j0:j1].unsqueeze(2).broadcast_to(
                    [128, j1 - j0, n, 2])

            def pe_chain(ps, off, taps, fd):
                segs = []
                c = off
                while c < off + fd:
                    c1 = min((c // 512 + 1) * 512, off + fd)
                    segs.append((c, c1))
                    c = c1
                for ti, (dap, src) in enumerate(taps):
                    for c0, c1 in segs:
                        nc.tensor.matmul(
                            ps[:, c0:c1], dap, src[:, c0 - off:c1 - off],
                            start=(ti == 0), stop=(ti == len(taps) - 1))

            NTT = len(SCHED)
            ioff = [0]
            roff = [0]
            for bt in SCHED:
                ioff.append(ioff[-1] + 7 * bt // 16)
                roff.append(roff[-1] + bt)

            def gather(t, xa, xb):
                bt = SCHED[t]
                na, nb = 4 * bt, 3 * bt
                c0 = ioff[t]
                sca = na // 16
                nc.gpsimd.dma_gather(
                    xa[:], tab.ap(), idx_sb[:, c0:c0 + sca],
                    na, na, EMBED, transpose=True, single_packet=False)
                nc.gpsimd.dma_gather(
                    xb[:], tab.ap(), idx_sb[:, c0 + sca:ioff[t + 1]],
                    nb, nb, EMBED, transpose=True, single_packet=False)

            xa0 = xpa.tile([128, JB, 4 * SCHED[0]], f16, tag="xa")
            xb0 = xpb.tile([128, JB, 3 * SCHED[0]], f16, tag="xb")
            xtiles = {0: (xa0, xb0)}
            gather(0, *xtiles[0])

            for t in range(NTT):
                B = SCHED[t]
                NA = 4 * B
                m1, a1 = (B, 0) if p1 == 0 else (0, B)
                m2, a2s = (B, 0) if p2 == 0 else (0, B)
                if t + 1 < NTT:
                    xan = xpa.tile([128, JB, 4 * SCHED[t + 1]], f16,
                                   tag="xa")
                    xbn = xpb.tile([128, JB, 3 * SCHED[t + 1]], f16,
                                   tag="xb")
                    xtiles[t + 1] = (xan, xbn)
                    gather(t + 1, *xtiles[t + 1])
                xta, xtb = xtiles.pop(t)

                H = JB // 2
                # ---- L1 (half-waves) ----
                az = azp.tile([128, JB, 6 * B], f16, tag="az")
                y1 = y1p.tile([128, JB, 6 * B], f16, tag="y1")
                for j0 in (0, H):
                    j1 = j0 + H
                    mA = NA - m1                 # mult cols from xta
                    nc.vector.tensor_tensor(
                        pair(az[:, j0:j1, 0:mA]),
                        pair(xta[:, j0:j1, m1:NA]),
                        kb(0, j0, j1, mA // 2), MUL)
                    nc.vector.tensor_tensor(
                        pair(az[:, j0:j1, mA:6 * B]),
                        pair(xtb[:, j0:j1, 0:6 * B - mA]),
                        kb(0, j0, j1, (6 * B - mA) // 2), MUL)
                    aA = NA - a1                 # add cols from xta
                    nc.vector.tensor_tensor(
                        az[:, j0:j1, 0:aA], az[:, j0:j1, 0:aA],
                        xta[:, j0:j1, a1:NA], ADD)
                    nc.vector.tensor_tensor(
                        az[:, j0:j1, aA:6 * B], az[:, j0:j1, aA:6 * B],
                        xtb[:, j0:j1, 0:6 * B - aA], ADD)
                    nc.scalar.activation(y1[:, j0:j1, :], az[:, j0:j1, :],
                                         SIG)

                # ---- L2 (half-waves) ----
                m = m2p.tile([128, JB, 5 * B], f16, tag="m2")
                y2 = y2p.tile([128, JB, 5 * B], f16, tag="y2")
                for j0 in (0, H):
                    j1 = j0 + H
                    nc.vector.tensor_tensor(
                        pair(m[:, j0:j1, :]),
                        pair(y1[:, j0:j1, m2:m2 + 5 * B]),
                        kb(1, j0, j1, 5 * B // 2), MUL)
                    nc.vector.tensor_tensor(
                        m[:, j0:j1, :], m[:, j0:j1, :],
                        y1[:, j0:j1, a2s:a2s + 5 * B], ADD)
                    nc.vector.tensor_tensor(
                        pair(m[:, j0:j1, :]), pair(m[:, j0:j1, :]),
                        kb(2, j0, j1, 5 * B // 2), MUL)
                    nc.scalar.activation(y2[:, j0:j1, :], m[:, j0:j1, :],
                                         SIG)

                # ---- L3: PE groups of 2 blocks ----
                y3 = y3p.tile([128, JB, 3 * B], f16, tag="y3")
                for g0 in range(0, JB, 2):
                    ps = pspool.tile([128, 2 * 3 * B], f32, tag="ps")
                    for gi, j in enumerate((g0, g0 + 1)):
                        pe_chain(ps, gi * 3 * B,
                                 [(dg(0, j, ti),
                                   y2[:, j, ti * B:ti * B + 3 * B])
                                  for ti in range(3)], 3 * B)
                    nc.scalar.activation(y3[:, g0:g0 + 2, :], ps[:], SIG)

                # ---- L4: PE blocks 0..PE4-1, DVE ratio-tts for the rest ----
                ps4 = pspool.tile([128, PE4 * B], f32, tag="ps")
                for j in range(PE4):
                    pe_chain(ps4, j * B,
                             [(dg(1, j, ti), y3[:, j, ti * B:(ti + 1) * B])
                              for ti in range(3)], B)
                y4 = y4p.tile([128, JB, B], f16, tag="y4")
                nc.scalar.activation(y4[:, 0:PE4, :], ps4[:], SIG)
                nd = JB - PE4
                c4 = c4p.tile([128, nd, B], f16, tag="c4")
                nc.vector.tensor_tensor(
                    pair(c4[:]),
                    pair(y3[:, PE4:JB, t4a * B:(t4a + 1) * B]),
                    kb(3, PE4, JB, B // 2), MUL)
                nc.vector.tensor_tensor(
                    c4[:], c4[:], y3[:, PE4:JB, p4 * B:(p4 + 1) * B], ADD)
                d4 = c4p.tile([128, nd, B], f16, tag="c4")
                nc.vector.tensor_tensor(
                    pair(d4[:]),
                    pair(y3[:, PE4:JB, t4b * B:(t4b + 1) * B]),
                    kb(4, PE4, JB, B // 2), MUL)
                nc.vector.tensor_tensor(c4[:], c4[:], d4[:], ADD)
                nc.vector.tensor_tensor(
                    pair(c4[:]), pair(c4[:]), kb(5, PE4, JB, B // 2), MUL)
                nc.scalar.activation(y4[:, PE4:JB, :], c4[:], SIG)

                half = JB // 2
                ob = JB * roff[t]
                nc.sync.dma_start(
                    out.ap()[:, ob:ob + half * B], y4[:, 0:half, :])
                nc.sync.dma_start(
                    out.ap()[:, ob + half * B:ob + JB * B],
                    y4[:, half:JB, :])

    nc.compile()
    _prog_cache["nc"] = nc
    return nc


def _pivot(k):
    k = np.asarray(k, np.float64)
    best, bp = None, 0
    for p in range(k.shape[0]):
        mx = np.abs(k / k[p:p + 1]).max()
        if best is None or mx < best:
            best, bp = mx, p
    return bp


def _pack_pairs(conv1, conv2, conv4, p1, p2, p4):
    c1 = np.asarray(conv1, np.float32)
    c2 = np.asarray(conv2, np.float32)
    c4 = np.asarray(conv4, np.float32)
    s1 = c1[1 - p1] / c1[p1]
    s2 = c2[1 - p2] / c2[p2]
    t4a, t4b = [t for t in range(3) if t != p4]
    vecs = (s1, s2, c2[p2], c4[t4a] / c4[p4], c4[t4b] / c4[p4], c4[p4])
    k = np.zeros((128, 6, JB, 2), np.float16)
    for w, vec in enumerate(vecs):
        v = vec.reshape(JB, 128).T
        k[:, w, :, 0] = v
        k[:, w, :, 1] = v
    return k.reshape(128, 6 * JB * 2)


def _pack_diags(conv3, conv4):
    c3 = np.asarray(conv3, np.float32).reshape(3, JB, 128)
    c4 = np.asarray(conv4, np.float32).reshape(3, JB, 128)
    d = np.zeros((128, NDIAG * 128), np.float32)
    s = 0
    for c in (c3, c4):
        for j in range(JB):
            for ti in range(3):
                np.fill_diagonal(d[:, s * 128:(s + 1) * 128], c[ti, j])
                s += 1
    return d.astype(np.float16)


def _make_idx(Xc):
    o = np.zeros((128, NT * SCOLS), np.int16)
    r0, c0 = 0, 0
    for bt in SCHED:
        rows = Xc[r0:r0 + bt, :]
        for sl in (slice(0, 4), slice(4, 7)):
            flat = rows[:, sl].T.reshape(-1)
            sc = len(flat) // 16
            wrap = flat.reshape(sc, 16).T.astype(np.int16)
            for m in range(8):
                o[16 * m:16 * m + 16, c0:c0 + sc] = wrap
            c0 += sc
        r0 += bt
    return o


def _unpermute(raw):
    raw = np.asarray(raw, np.float32)
    o = np.empty((BCORE, EMBED), np.float32)
    r0 = 0
    for bt in SCHED:
        a = raw[:, JB * r0:JB * (r0 + bt)].reshape(128, JB, bt)
        o[r0:r0 + bt] = a.transpose(2, 1, 0).reshape(bt, EMBED)
        r0 += bt
    return o


def run(X, emb, conv1, conv2, conv3, conv4, **spmd_kwargs):
    X = np.asarray(X)
    emb = np.asarray(emb, np.float32)
    c1 = np.asarray(conv1, np.float32)
    p1, p2, p4 = _prog_cache.setdefault(
        "pivots", (_pivot(conv1), _pivot(conv2), _pivot(conv4)))
    nc = _build_program()

    table = (emb * c1[p1][None, :]).astype(np.float16)
    kpack = _pack_pairs(conv1, conv2, conv4, p1, p2, p4)
    dpack = _pack_diags(conv3, conv4)

    in_maps = []
    for c in range(NCORES):
        Xc = X[c * BCORE:(c + 1) * BCORE]
        in_maps.append({"tab": table, "idx": _make_idx(Xc), "kp": kpack,
                        "diags": dpack})

    res = bass_utils.run_bass_kernel_spmd(nc, in_maps,
                                          core_ids=list(range(NCORES)),
                                          **spmd_kwargs)
    o = np.concatenate(
        [_unpermute(res.results[c]["out"]) for c in range(NCORES)], axis=0)
    return o, res


def kernel(X, emb, conv1, conv2, conv3, conv4):
    o, _ = run(X, emb, conv1, conv2, conv3, conv4)
    return o


# revision 6
# speedup vs baseline: 1.0955x; 1.0239x over previous
"""v4 Trainium2 Bass kernel for nn_CSM_62216896250023 (dense_cnn).

One merged sigmoid ACT instruction per layer per tile (7-8 ACT/tile).
Single SWDGE gather per tile from a host-prescaled table (emb * pivot
tap of conv1). Conv taps applied with pair-broadcast tensor_tensor
(2x DVE fast mode, per-channel constants as packed [.,2] pairs):
  L1: a = x'[mult]*S1 (tt), Z1 = a += x'[add] (tt, in-place)
  L2: m = Y1[mult]*S2, m += Y1[add], Z2 = m *= Kpivot (DVE j0-3,
      Pool j4-7)
  L3: PE true-diag 3-tap chains, psum groups {2,2,2,2}
  L4: PE true-diag chains -> one psum group
All fp16; output fp16 -> host fp32.
"""
import numpy as np

import concourse.bacc as bacc
import concourse.tile as tile
import concourse.bass_utils as bass_utils
from concourse import mybir

VOCAB, EMBED, BATCH, SEQ = 32000, 1024, 16384, 7
NCORES = 8
BCORE = BATCH // NCORES          # 2048
BT = 256
NT = BCORE // BT                 # 8 tiles
# ramp/drain compression: small tiles first and last so the pipeline
# fills and drains quickly; gather rows must stay multiples of 128
SCHED = (128, 128, 256, 256, 256, 256, 256, 256, 128, 128)
assert sum(SCHED) == BCORE
JB = EMBED // 128                # 8
NIDX = SEQ * BT                  # 1792
SCOLS = NIDX // 16               # 112
NDIAG = JB * 3 * 2               # 48 (L3 + L4)
POOL_KLAST_J0 = 4                # j4-7 L2 k-pivot mult on Pool

_prog_cache = {}


def _build_program():
    if "nc" in _prog_cache:
        return _prog_cache["nc"]
    f32, f16, i16 = mybir.dt.float32, mybir.dt.float16, mybir.dt.int16
    SIG = mybir.ActivationFunctionType.Sigmoid
    MUL, ADD = mybir.AluOpType.mult, mybir.AluOpType.add

    nc = bacc.Bacc("TRN2", target_bir_lowering=False, debug=False)
    tab = nc.dram_tensor("tab", [VOCAB, EMBED], f16, kind="ExternalInput")
    idx = nc.dram_tensor("idx", [128, NT * SCOLS], i16, kind="ExternalInput")
    # pair consts: [s1, s2, k2piv, s4a, s4b, k4piv] each [128, JB, 2]
    kp = nc.dram_tensor("kp", [128, 6 * JB * 2], f16, kind="ExternalInput")
    # mult/add slice offsets depend on host pivot choice -> pass via const
    # tensor is not possible for slicing; host guarantees pivot choice at
    # build time through module-level PIV values set before compile.
    diags = nc.dram_tensor("diags", [128, NDIAG * 128], f16,
                           kind="ExternalInput")
    scl = nc.dram_tensor("scl", [128, JB], f32, kind="ExternalInput")
    out = nc.dram_tensor("out", [128, NT * JB * BT], f16,
                         kind="ExternalOutput")

    p1, p2, p4 = _prog_cache["pivots"]
    t4a, t4b = [t for t in range(3) if t != p4]  # non-pivot taps of conv4
    PE4 = 4                                       # L4 blocks on PE; rest DVE

    with tile.TileContext(nc) as tc:
        with tc.tile_pool(name="const", bufs=1) as cpool, \
             tc.tile_pool(name="xpa", bufs=2) as xpa, \
             tc.tile_pool(name="xpb", bufs=2) as xpb, \
             tc.tile_pool(name="az", bufs=1) as azp, \
             tc.tile_pool(name="y1", bufs=1) as y1p, \
             tc.tile_pool(name="m2", bufs=1) as m2p, \
             tc.tile_pool(name="y2", bufs=1) as y2p, \
             tc.tile_pool(name="y3", bufs=2) as y3p, \
             tc.tile_pool(name="c4", bufs=2) as c4p, \
             tc.tile_pool(name="y4", bufs=1) as y4p, \
             tc.tile_pool(name="ps", bufs=2, space="PSUM") as pspool:

            idx_sb = cpool.tile([128, NT * SCOLS], i16)
            nc.sync.dma_start(idx_sb[:], idx.ap())
            kp_sb = cpool.tile([128, 6, JB, 2], f16)
            nc.sync.dma_start(kp_sb[:], kp.ap())
            d_sb = cpool.tile([128, NDIAG * 128], f16)
            nc.sync.dma_start(d_sb[:], diags.ap())
            scl_sb = cpool.tile([128, JB], f32)
            nc.sync.dma_start(scl_sb[:], scl.ap())

            def dg(layer, j, ti):          # layer 0 = L3, 1 = L4
                s = layer * JB * 3 + j * 3 + ti
                return d_sb[:, s * 128:(s + 1) * 128]

            def pair(ap3):
                return ap3.rearrange("p j (a b) -> p j a b", b=2)

            def kb(which, j0, j1, n):
                return kp_sb[:, which, j0:j1].unsqueeze(2).broadcast_to(
                    [128, j1 - j0, n, 2])

            def pe_chain(ps, off, taps, fd):
                segs = []
                c = off
                while c < off + fd:
                    c1 = min((c // 512 + 1) * 512, off + fd)
                    segs.append((c, c1))
                    c = c1
                for ti, (dap, src) in enumerate(taps):
                    for c0, c1 in segs:
                        nc.tensor.matmul(
                            ps[:, c0:c1], dap, src[:, c0 - off:c1 - off],
                            start=(ti == 0), stop=(ti == len(taps) - 1))

            NTT = len(SCHED)
            ioff = [0]
            roff = [0]
            for bt in SCHED:
                ioff.append(ioff[-1] + 7 * bt // 16)
                roff.append(roff[-1] + bt)

            def gather(t, xa, xb):
                bt = SCHED[t]
                na, nb = 4 * bt, 3 * bt
                c0 = ioff[t]
                sca = na // 16
                nc.gpsimd.dma_gather(
                    xa[:], tab.ap(), idx_sb[:, c0:c0 + sca],
                    na, na, EMBED, transpose=True, single_packet=False)
                nc.gpsimd.dma_gather(
                    xb[:], tab.ap(), idx_sb[:, c0 + sca:ioff[t + 1]],
                    nb, nb, EMBED, transpose=True, single_packet=False)

            xa0 = xpa.tile([128, JB, 4 * SCHED[0]], f16, tag="xa")
            xb0 = xpb.tile([128, JB, 3 * SCHED[0]], f16, tag="xb")
            xtiles = {0: (xa0, xb0)}
            gather(0, *xtiles[0])

            for t in range(NTT):
                B = SCHED[t]
                NA = 4 * B
                m1, a1 = (B, 0) if p1 == 0 else (0, B)
                m2, a2s = (B, 0) if p2 == 0 else (0, B)
                if t + 1 < NTT:
                    xan = xpa.tile([128, JB, 4 * SCHED[t + 1]], f16,
                                   tag="xa")
                    xbn = xpb.tile([128, JB, 3 * SCHED[t + 1]], f16,
                                   tag="xb")
                    xtiles[t + 1] = (xan, xbn)
                    gather(t + 1, *xtiles[t + 1])
                xta, xtb = xtiles.pop(t)

                H = JB // 2
                # ---- L1 (half-waves) ----
                az = azp.tile([128, JB, 6 * B], f16, tag="az")
                y1 = y1p.tile([128, JB, 6 * B], f16, tag="y1")
                for j0 in (0, H):
                    j1 = j0 + H
                    mA = NA - m1                 # mult cols from xta
                    nc.vector.tensor_tensor(
                        pair(az[:, j0:j1, 0:mA]),
                        pair(xta[:, j0:j1, m1:NA]),
                        kb(0, j0, j1, mA // 2), MUL)
                    nc.vector.tensor_tensor(
                        pair(az[:, j0:j1, mA:6 * B]),
                        pair(xtb[:, j0:j1, 0:6 * B - mA]),
                        kb(0, j0, j1, (6 * B - mA) // 2), MUL)
                    aA = NA - a1                 # add cols from xta
                    nc.vector.tensor_tensor(
                        az[:, j0:j1, 0:aA], az[:, j0:j1, 0:aA],
                        xta[:, j0:j1, a1:NA], ADD)
                    nc.vector.tensor_tensor(
                        az[:, j0:j1, aA:6 * B], az[:, j0:j1, aA:6 * B],
                        xtb[:, j0:j1, 0:6 * B - aA], ADD)
                    nc.scalar.activation(y1[:, j0:j1, :], az[:, j0:j1, :],
                                         SIG)

                # ---- L2 (half-waves) ----
                m = m2p.tile([128, JB, 5 * B], f16, tag="m2")
                y2 = y2p.tile([128, JB, 5 * B], f16, tag="y2")
                for j0 in (0, H):
                    j1 = j0 + H
                    nc.vector.tensor_tensor(
                        pair(m[:, j0:j1, :]),
                        pair(y1[:, j0:j1, m2:m2 + 5 * B]),
                        kb(1, j0, j1, 5 * B // 2), MUL)
                    nc.vector.tensor_tensor(
                        m[:, j0:j1, :], m[:, j0:j1, :],
                        y1[:, j0:j1, a2s:a2s + 5 * B], ADD)
                    # k-pivot for the first 2 blocks on DVE; the other
                    # 2 ride the ACT per-partition scale (sigmoid(k*u))
                    nc.vector.tensor_tensor(
                        pair(m[:, j0:j0 + 2, :]), pair(m[:, j0:j0 + 2, :]),
                        kb(2, j0, j0 + 2, 5 * B // 2), MUL)
                    nc.scalar.activation(y2[:, j0:j0 + 2, :],
                                         m[:, j0:j0 + 2, :], SIG)
                    for js in (j0 + 2, j0 + 3):
                        nc.scalar.activation(y2[:, js, :], m[:, js, :], SIG,
                                             scale=scl_sb[:, js:js + 1])

                # ---- L3: PE groups of 2 blocks ----
                y3 = y3p.tile([128, JB, 3 * B], f16, tag="y3")
                for g0 in range(0, JB, 2):
                    ps = pspool.tile([128, 2 * 3 * B], f32, tag="ps")
                    for gi, j in enumerate((g0, g0 + 1)):
                        pe_chain(ps, gi * 3 * B,
                                 [(dg(0, j, ti),
                                   y2[:, j, ti * B:ti * B + 3 * B])
                                  for ti in range(3)], 3 * B)
                    nc.scalar.activation(y3[:, g0:g0 + 2, :], ps[:], SIG)

                # ---- L4: PE blocks 0..PE4-1, DVE ratio-tts for the rest ----
                ps4 = pspool.tile([128, PE4 * B], f32, tag="ps")
                for j in range(PE4):
                    pe_chain(ps4, j * B,
                             [(dg(1, j, ti), y3[:, j, ti * B:(ti + 1) * B])
                              for ti in range(3)], B)
                y4 = y4p.tile([128, JB, B], f16, tag="y4")
                nc.scalar.activation(y4[:, 0:PE4, :], ps4[:], SIG)
                nd = JB - PE4
                c4 = c4p.tile([128, nd, B], f16, tag="c4")
                nc.vector.tensor_tensor(
                    pair(c4[:]),
                    pair(y3[:, PE4:JB, t4a * B:(t4a + 1) * B]),
                    kb(3, PE4, JB, B // 2), MUL)
                nc.vector.tensor_tensor(
                    c4[:], c4[:], y3[:, PE4:JB, p4 * B:(p4 + 1) * B], ADD)
                d4 = c4p.tile([128, nd, B], f16, tag="c4")
                nc.vector.tensor_tensor(
                    pair(d4[:]),
                    pair(y3[:, PE4:JB, t4b * B:(t4b + 1) * B]),
                    kb(4, PE4, JB, B // 2), MUL)
                nc.vector.tensor_tensor(c4[:], c4[:], d4[:], ADD)
                nc.vector.tensor_tensor(
                    pair(c4[:]), pair(c4[:]), kb(5, PE4, JB, B // 2), MUL)
                nc.scalar.activation(y4[:, PE4:JB, :], c4[:], SIG)

                half = JB // 2
                ob = JB * roff[t]
                nc.sync.dma_start(
                    out.ap()[:, ob:ob + half * B], y4[:, 0:half, :])
                nc.sync.dma_start(
                    out.ap()[:, ob + half * B:ob + JB * B],
                    y4[:, half:JB, :])

    nc.compile()
    _prog_cache["nc"] = nc
    return nc


def _pivot(k):
    k = np.asarray(k, np.float64)
    best, bp = None, 0
    for p in range(k.shape[0]):
        mx = np.abs(k / k[p:p + 1]).max()
        if best is None or mx < best:
            best, bp = mx, p
    return bp


def _pack_pairs(conv1, conv2, conv4, p1, p2, p4):
    c1 = np.asarray(conv1, np.float32)
    c2 = np.asarray(conv2, np.float32)
    c4 = np.asarray(conv4, np.float32)
    s1 = c1[1 - p1] / c1[p1]
    s2 = c2[1 - p2] / c2[p2]
    t4a, t4b = [t for t in range(3) if t != p4]
    vecs = (s1, s2, c2[p2], c4[t4a] / c4[p4], c4[t4b] / c4[p4], c4[p4])
    k = np.zeros((128, 6, JB, 2), np.float16)
    for w, vec in enumerate(vecs):
        v = vec.reshape(JB, 128).T
        k[:, w, :, 0] = v
        k[:, w, :, 1] = v
    return k.reshape(128, 6 * JB * 2)


def _pack_diags(conv3, conv4):
    c3 = np.asarray(conv3, np.float32).reshape(3, JB, 128)
    c4 = np.asarray(conv4, np.float32).reshape(3, JB, 128)
    d = np.zeros((128, NDIAG * 128), np.float32)
    s = 0
    for c in (c3, c4):
        for j in range(JB):
            for ti in range(3):
                np.fill_diagonal(d[:, s * 128:(s + 1) * 128], c[ti, j])
                s += 1
    return d.astype(np.float16)


def _make_idx(Xc):
    o = np.zeros((128, NT * SCOLS), np.int16)
    r0, c0 = 0, 0
    for bt in SCHED:
        rows = Xc[r0:r0 + bt, :]
        for sl in (slice(0, 4), slice(4, 7)):
            flat = rows[:, sl].T.reshape(-1)
            sc = len(flat) // 16
            wrap = flat.reshape(sc, 16).T.astype(np.int16)
            for m in range(8):
                o[16 * m:16 * m + 16, c0:c0 + sc] = wrap
            c0 += sc
        r0 += bt
    return o


def _unpermute(raw):
    raw = np.asarray(raw, np.float32)
    o = np.empty((BCORE, EMBED), np.float32)
    r0 = 0
    for bt in SCHED:
        a = raw[:, JB * r0:JB * (r0 + bt)].reshape(128, JB, bt)
        o[r0:r0 + bt] = a.transpose(2, 1, 0).reshape(bt, EMBED)
        r0 += bt
    return o


def run(X, emb, conv1, conv2, conv3, conv4, **spmd_kwargs):
    X = np.asarray(X)
    emb = np.asarray(emb, np.float32)
    c1 = np.asarray(conv1, np.float32)
    p1, p2, p4 = _prog_cache.setdefault(
        "pivots", (_pivot(conv1), _pivot(conv2), _pivot(conv4)))
    nc = _build_program()

    table = (emb * c1[p1][None, :]).astype(np.float16)
    kpack = _pack_pairs(conv1, conv2, conv4, p1, p2, p4)
    dpack = _pack_diags(conv3, conv4)

    c2 = np.asarray(conv2, np.float32)
    sclpack = np.ascontiguousarray(c2[p2].reshape(JB, 128).T)
    in_maps = []
    for c in range(NCORES):
        Xc = X[c * BCORE:(c + 1) * BCORE]
        in_maps.append({"tab": table, "idx": _make_idx(Xc), "kp": kpack,
                        "diags": dpack, "scl": sclpack})

    res = bass_utils.run_bass_kernel_spmd(nc, in_maps,
                                          core_ids=list(range(NCORES)),
                                          **spmd_kwargs)
    o = np.concatenate(
        [_unpermute(res.results[c]["out"]) for c in range(NCORES)], axis=0)
    return o, res


def kernel(X, emb, conv1, conv2, conv3, conv4):
    o, _ = run(X, emb, conv1, conv2, conv3, conv4)
    return o
